# revision 1
# baseline (speedup 1.0000x reference)
# MoE routing kernel for Trainium2 (Bass/Tile), SPMD over 8 NeuronCores.
#
# Reference computation (B=4, T=2048, D=H=1024, V=8, L=4):
#   h      = gelu(einsum("btd,vdh->btvh", X, W1) + b1)
#   outs   = einsum("btvh,vhk->btvk", h, W2) + b2
#   w      = softmax(op_logits, axis=-1)            # [B, L, V]
#   result = einsum("blv,btvh->bth", w, outs) / L
#
# Strategy:
#   - Host: softmax + mean over L -> wbar[B, V]; fold b2 into a single
#     per-batch combined bias cbias[b] = sum_v wbar[b,v] * b2[v].
#   - Data parallel over tokens: core c owns tokens [c*1024, (c+1)*1024).
#     Each 1024-token shard lies inside a single batch row b, so wbar/cbias
#     are per-core constants (shipped as data => one SPMD program).
#   - Per core, per expert v:
#       MM1:  pre1^T[h, t] = sum_d W1[v][d, h]^T-free X^T[d, t]   (PE)
#       gelu: h_sb[h, t] = Gelu(pre1 + b1[v][h])                  (ACT, PSUM->SBUF)
#       MM2:  out[t, k]  = sum_h h_sb[h, t]-as-lhsT W2[v][h, k]   (PE)
#       acc:  out_acc[t, k] = wbar[v] * out + (cbias | out_acc)   (DVE)
#   - X is pre-transposed on host to [D, BT] so every matmul operand is
#     naturally contraction-major; no on-device transposes anywhere.

import os

import numpy as np
import ml_dtypes

import concourse.bass as bass
import concourse.mybir as mybir
import concourse.tile as tile
from concourse import bacc
from concourse.bass_utils import run_bass_kernel_spmd

N_CORES = 8
P = 128

_DT_MAP = {
    "bf16": mybir.dt.bfloat16,
    "f32r": mybir.dt.float32r,
    "f32": mybir.dt.float32,
}
_NP_DT_MAP = {
    "bf16": ml_dtypes.bfloat16,
    "f32r": np.float32,
    "f32": np.float32,
}


def build_moe_core_program(TC, D, H, V, mode="bf16", act="gelu"):
    """One NeuronCore's program: TC tokens, full V experts."""
    act_func = {
        "gelu": mybir.ActivationFunctionType.Gelu,
        "tanh": mybir.ActivationFunctionType.Tanh,  # sim-only (CoreSim lacks Gelu)
    }[act]
    DT = _DT_MAP[mode]
    f32 = mybir.dt.float32
    DC = D // P          # contraction chunks for MM1
    HC = H // P          # contraction chunks for MM2
    NT = min(512, TC)    # MM1 moving free dim (tokens)
    NK = min(512, H)     # MM2 moving free dim (output cols)
    TT = TC // P         # token tiles of 128

    # Bacc (not plain Bass): its finalize() runs generate_event_semaphores,
    # which splits multi-sem waits — TRN2 allows max 1 wait per instruction.
    nc = bacc.Bacc(trn_type="TRN2")
    x_t = nc.declare_dram_parameter("x_t", [D, TC], DT, isOutput=False)
    w1 = nc.declare_dram_parameter("w1", [V, D, H], DT, isOutput=False)
    w2 = nc.declare_dram_parameter("w2", [V, H, H], DT, isOutput=False)
    b1t = nc.declare_dram_parameter("b1t", [H, V], f32, isOutput=False)
    wbar = nc.declare_dram_parameter("wbar", [P, V], f32, isOutput=False)
    cbias = nc.declare_dram_parameter("cbias", [P, H], f32, isOutput=False)
    out = nc.declare_dram_parameter("out", [TC, H], f32, isOutput=True)

    with tile.TileContext(nc) as tc:
        with (
            tc.tile_pool(name="const", bufs=1) as cpool,
            tc.tile_pool(name="w1p", bufs=2) as w1p,
            tc.tile_pool(name="w2p", bufs=2) as w2p,
            tc.tile_pool(name="hbuf", bufs=1) as hpool,
            tc.tile_pool(name="accp", bufs=1) as accp,
            tc.tile_pool(name="ps1", bufs=4, space="PSUM") as ps1,
            tc.tile_pool(name="ps2", bufs=4, space="PSUM") as ps2,
        ):
            NTH = TC // NT  # token halves
            # Small constants first — negligible bandwidth, needed by the
            # first gelu / accumulate.
            b1_sb = cpool.tile([P, HC, V], f32)
            nc.sync.dma_start(out=b1_sb, in_=b1t.rearrange("(hc p) v -> p hc v", p=P))
            wbar_sb = cpool.tile([P, V], f32)
            nc.sync.dma_start(out=wbar_sb, in_=wbar[:])
            cbias_sb = cpool.tile([P, H], f32)
            nc.sync.dma_start(out=cbias_sb, in_=cbias[:])
            # per-tt output accumulators: each tt's store depends only on its
            # own tile, so final DMAs overlap the last expert's compute.
            out_tiles = [
                accp.tile([P, H], f32, tag=f"acc{tt}", name=f"acc{tt}")
                for tt in range(TT)
            ]
            out_r = out.rearrange("(tt p) k -> p tt k", p=P)

            # x per-dc tiles (2KB DMA lines), unchained — they and w1[0] are
            # the startup-critical set and stream concurrently at full fanout.
            x_tiles = []
            for dc in range(DC):
                xt = cpool.tile([P, TC], DT, tag=f"x{dc}", name=f"x{dc}")
                nc.sync.dma_start(out=xt, in_=x_t[dc * P:(dc + 1) * P, :])
                x_tiles.append(xt)

            # Later weight blocks are chained (2MB granularity) with explicit
            # sync deps: each block's descriptors only enqueue after the
            # previous block's transfer completes. The HW DGE engines service
            # queues round-robin, so unchained later blocks would steal
            # bandwidth from the startup-critical x + w1[0] transfers. (Finer,
            # 256KB-link chaining was tried and starves PE: each sem-gated
            # link adds ~3us of issue latency, throttling the weight stream.)
            prev_block = None  # last DMA of the previous weight block

            def _dep_on_prev(dma):
                if prev_block is not None:
                    bass._add_dep_helper(
                        dma.ins, prev_block.ins, sync=True,
                        reason="serialize weight-block DMA issue",
                    )

            for v in range(V):
                # w1 per-dc tiles: first expert's matmuls begin after the
                # first ~256KB chunks instead of the full 2MB block.
                w1_tiles = []
                for dc in range(DC):
                    w1t = w1p.tile([P, H], DT, tag=f"w1_{dc}", name=f"w1_{dc}")
                    w1dma = nc.sync.dma_start(
                        out=w1t, in_=w1[v, dc * P:(dc + 1) * P, :]
                    )
                    _dep_on_prev(w1dma)
                    w1_tiles.append(w1t)
                prev_block = w1dma
                h_sb = hpool.tile([P, HC, TC], DT, tag="h")

                # MM1 + gelu: produce h-major activations h_sb[h, t]
                for hc in range(HC):
                    for th in range(NTH):
                        p1 = ps1.tile([P, NT], f32, tag="p1")
                        for dc in range(DC):
                            nc.tensor.matmul(
                                p1,
                                w1_tiles[dc][:, hc * P:(hc + 1) * P],
                                x_tiles[dc][:, th * NT:(th + 1) * NT],
                                start=(dc == 0),
                                stop=(dc == DC - 1),
                            )
                        nc.scalar.activation(
                            h_sb[:, hc, th * NT:(th + 1) * NT],
                            p1,
                            act_func,
                            bias=b1_sb[:, hc, v:v + 1],
                        )

                # w2 emitted after MM1 + chained, so its transfer can't steal
                # bandwidth from the startup-critical x/w1[0] loads.
                w2_sb = w2p.tile([P, HC, H], DT, tag="w2")
                w2dma = nc.sync.dma_start(
                    out=w2_sb, in_=w2[v].rearrange("(hc p) k -> p hc k", p=P)
                )
                _dep_on_prev(w2dma)
                prev_block = w2dma

                # MM2 + weighted accumulate into out_tiles[tt][t, k]
                for tt in range(TT):
                    for kc in range(H // NK):
                        p2 = ps2.tile([P, NK], f32, tag="p2")
                        for hc in range(HC):
                            nc.tensor.matmul(
                                p2,
                                h_sb[:, hc, tt * P:(tt + 1) * P],
                                w2_sb[:, hc, kc * NK:(kc + 1) * NK],
                                start=(hc == 0),
                                stop=(hc == HC - 1),
                            )
                        ksl = slice(kc * NK, (kc + 1) * NK)
                        in1 = cbias_sb[:, ksl] if v == 0 else out_tiles[tt][:, ksl]
                        nc.vector.scalar_tensor_tensor(
                            out=out_tiles[tt][:, ksl],
                            in0=p2,
                            scalar=wbar_sb[:, v:v + 1],
                            in1=in1,
                            op0=mybir.AluOpType.mult,
                            op1=mybir.AluOpType.add,
                        )
                    if v == V - 1:
                        # store this tt as soon as its accumulation finishes
                        nc.sync.dma_start(out=out_r[:, tt, :], in_=out_tiles[tt][:])
    nc.finalize()  # Bacc: runs wait-splitting + reg alloc passes
    return nc


_prog_cache = {}


def _get_program(mode, TC, D, H, V):
    key = (mode, TC, D, H, V)
    if key not in _prog_cache:
        _prog_cache[key] = build_moe_core_program(TC, D, H, V, mode=mode)
    return _prog_cache[key]


def host_prep(op_logits, token_feats, W1, b1, W2, b2, mode):
    """Shared host-side preprocessing: softmax folding, transpose, cast, shard."""
    op_logits = np.asarray(op_logits, dtype=np.float32)
    token_feats = np.asarray(token_feats, dtype=np.float32)
    W1 = np.asarray(W1, dtype=np.float32)
    b1 = np.asarray(b1, dtype=np.float32)
    W2 = np.asarray(W2, dtype=np.float32)
    b2 = np.asarray(b2, dtype=np.float32)

    B, T, D = token_feats.shape
    V, _, H = W1.shape
    BT = B * T
    TC = BT // N_CORES

    lg = op_logits.astype(np.float64)
    e = np.exp(lg - lg.max(axis=-1, keepdims=True))
    w = e / e.sum(axis=-1, keepdims=True)
    wbar = w.mean(axis=1)                       # [B, V], includes the 1/L
    cbias = wbar @ b2.astype(np.float64)        # [B, H]

    np_dt = _NP_DT_MAP[mode]
    x_t = np.ascontiguousarray(token_feats.reshape(BT, D).T).astype(np_dt)
    w1c = np.ascontiguousarray(W1.astype(np_dt))
    w2c = np.ascontiguousarray(W2.astype(np_dt))
    b1t = np.ascontiguousarray(b1.T.astype(np.float32))

    in_maps = []
    for c in range(N_CORES):
        bc = (c * TC) // T
        in_maps.append({
            "x_t": np.ascontiguousarray(x_t[:, c * TC:(c + 1) * TC]),
            "w1": w1c,
            "w2": w2c,
            "b1t": b1t,
            "wbar": np.ascontiguousarray(
                np.broadcast_to(wbar[bc].astype(np.float32), (P, V))
            ),
            "cbias": np.ascontiguousarray(
                np.broadcast_to(cbias[bc].astype(np.float32), (P, H))
            ),
        })
    return in_maps, (B, T, D, H, V, TC)


LAST_RESULTS = None


def kernel(op_logits, token_feats, W1, b1, W2, b2):
    global LAST_RESULTS
    mode = os.environ.get("MOE_DTYPE", "bf16")
    in_maps, (B, T, D, H, V, TC) = host_prep(
        op_logits, token_feats, W1, b1, W2, b2, mode
    )
    nc = _get_program(mode, TC, D, H, V)
    res = run_bass_kernel_spmd(
        nc,
        in_maps,
        list(range(N_CORES)),
        trace=os.environ.get("MOE_TRACE", "0") == "1",
    )
    LAST_RESULTS = res
    outs = [res.results[c]["out"] for c in range(N_CORES)]
    return np.concatenate(outs, axis=0).reshape(B, T, H).astype(np.float32)



# revision 2
# speedup vs baseline: 1.1533x; 1.1533x over previous
# MoE routing kernel for Trainium2 (Bass/Tile), SPMD over 8 NeuronCores.
#
# Reference computation (B=4, T=2048, D=H=1024, V=8, L=4):
#   h      = gelu(einsum("btd,vdh->btvh", X, W1) + b1)
#   outs   = einsum("btvh,vhk->btvk", h, W2) + b2
#   w      = softmax(op_logits, axis=-1)            # [B, L, V]
#   result = einsum("blv,btvh->bth", w, outs) / L
#
# Strategy (v2, mixed precision):
#   - Host: softmax + mean over L -> wbar[B, V]; fold b2 into cbias = wbar@b2.
#   - Data parallel over tokens: core c owns tokens [c*1024, (c+1)*1024), all
#     inside one batch row, so wbar/cbias are per-core constants.
#   - wbar is folded into W2 on host (W2 * wbar_v * 128), so MM2 accumulates
#     across ALL experts directly in PSUM; the single post-op per output tile
#     is out = psum/128 + cbias.
#   - Mixed precision expert slots: the program has a fixed list of 8 slots,
#     each (mm1, mm2) in {bf16, fp8e4}^2. fp8 matmuls use DoubleRow perf mode
#     (256-deep contraction per instruction = 2x bf16 throughput; verified on
#     HW). Per core, experts are assigned to slots by descending wbar (heavy
#     experts -> bf16 slots), which keeps the softmax-weighted quantization
#     error under the harness gate: sim says 5xbb+3x88 -> rel 1.84e-2.
#   - fp8 scaling: W1*32 quantized, descaled by the gelu activation's input
#     scale (1/32); X and h quantized at natural scale (subnormal loss of
#     tiny elements contributes negligibly to 1024-deep dot products).
#   - Loop structure per core: two 512-token halves; per half one MM1 phase
#     (h for all 8 slots resident in SBUF) then two MM2 column passes (kc)
#     accumulating over slots into 4 persistent PSUM banks; ps1 rotates in 3
#     other banks. Weights stream in consumption order on a single DMA chain.

import os

import numpy as np
import ml_dtypes

import concourse.bass as bass
import concourse.mybir as mybir
import concourse.tile as tile
from concourse import bacc
from concourse.bass_utils import run_bass_kernel_spmd

N_CORES = 8
P = 128
C_W2 = 128.0  # global PSUM scale carried by the folded W2
W1_SCALE = 32.0  # fp8 W1 pre-scale, descaled in the gelu activation

BF16 = mybir.dt.bfloat16
FP8 = mybir.dt.float8e4
F32 = mybir.dt.float32
NP_BF16 = ml_dtypes.bfloat16
NP_FP8 = ml_dtypes.float8_e4m3

DEFAULT_SLOTS = "88,88,88,bb,bb,bb,bb,bb"


def build_program(slots, TC, D, H):
    """One NeuronCore's program: TC tokens, len(slots) expert slots."""
    NS = len(slots)
    DC, HC = D // P, H // P
    DQ, HQ = DC // 2, HC // 2
    T2 = 512            # tokens per half
    NH = TC // T2       # halves
    TT = T2 // P        # 128-token tiles per half
    KC = 2              # 512-col MM2 passes
    KW = H // KC
    n1b = sum(s[0] == "b" for s in slots)
    n18 = NS - n1b
    n2b = sum(s[1] == "b" for s in slots)
    n28 = NS - n2b

    nc = bacc.Bacc(trn_type="TRN2")
    dram = {}
    if n1b:
        dram["xb"] = nc.declare_dram_parameter("xb", [D, TC], BF16, isOutput=False)
        dram["w1b"] = nc.declare_dram_parameter(
            "w1b", [n1b, D, H], BF16, isOutput=False
        )
    if n18:
        dram["x8"] = nc.declare_dram_parameter(
            "x8", [DQ, P, 2, TC], FP8, isOutput=False
        )
        dram["w18"] = nc.declare_dram_parameter(
            "w18", [n18, DQ, P, 2, H], FP8, isOutput=False
        )
    if n2b:
        dram["w2b"] = nc.declare_dram_parameter(
            "w2b", [n2b, KC, P, HC, KW], BF16, isOutput=False
        )
    if n28:
        dram["w28"] = nc.declare_dram_parameter(
            "w28", [n28, KC, P, HC, KW], FP8, isOutput=False
        )
    b1t = nc.declare_dram_parameter("b1t", [H, NS], F32, isOutput=False)
    cbias = nc.declare_dram_parameter("cbias", [P, H], F32, isOutput=False)
    out = nc.declare_dram_parameter("out", [TC, H], F32, isOutput=True)

    # per-slot index into the per-precision weight arrays
    i1 = {}
    c1b = c18 = 0
    for s, sl in enumerate(slots):
        if sl[0] == "b":
            i1[s] = c1b
            c1b += 1
        else:
            i1[s] = c18
            c18 += 1
    i2 = {}
    c2b = c28 = 0
    for s, sl in enumerate(slots):
        if sl[1] == "b":
            i2[s] = c2b
            c2b += 1
        else:
            i2[s] = c28
            c28 += 1

    with tile.TileContext(nc) as tc:
        with (
            tc.tile_pool(name="const", bufs=1) as cpool,
            tc.tile_pool(name="w1bp", bufs=2) as w1bp,
            tc.tile_pool(name="w18p", bufs=2) as w18p,
            tc.tile_pool(name="w2p", bufs=2) as w2p,
            tc.tile_pool(name="hbuf", bufs=1) as hpool,
            tc.tile_pool(name="outp", bufs=3) as outp,
            tc.tile_pool(name="ps1", bufs=3, space="PSUM") as ps1,
            tc.tile_pool(name="pout", bufs=1, space="PSUM") as poutp,
        ):
            b1_sb = cpool.tile([P, HC, NS], F32)
            nc.sync.dma_start(out=b1_sb, in_=b1t.rearrange("(hc p) v -> p hc v", p=P))
            cbias_sb = cpool.tile([P, H], F32)
            nc.sync.dma_start(out=cbias_sb, in_=cbias[:])

            # x tiles: unchained, startup-critical, stream at full fanout
            xb_t, x8_t = [], []
            if n1b:
                for dc in range(DC):
                    t = cpool.tile([P, TC], BF16, tag=f"xb{dc}", name=f"xb{dc}")
                    nc.sync.dma_start(out=t, in_=dram["xb"][dc * P:(dc + 1) * P, :])
                    xb_t.append(t)
            if n18:
                for dq in range(DQ):
                    t = cpool.tile([P, 2, TC], FP8, tag=f"x8{dq}", name=f"x8{dq}")
                    nc.sync.dma_start(out=t, in_=dram["x8"][dq])
                    x8_t.append(t)

            pout = [
                poutp.tile([P, KW], F32, tag=f"po{tt}", name=f"po{tt}")
                for tt in range(TT)
            ]
            out_r = out.rearrange("(tb p) k -> p tb k", p=P)

            # Weight blocks are chained (block granularity) so later transfers
            # can't steal bandwidth from the startup-critical x + first-slot
            # loads. Each block's DMAs depend on the previous block's last DMA.
            prev_block = [None]

            def chain(dma):
                if prev_block[0] is not None:
                    bass._add_dep_helper(
                        dma.ins, prev_block[0].ins, sync=True,
                        reason="serialize weight-block DMA issue",
                    )

            for half in range(NH):
                tsl = slice(half * T2, (half + 1) * T2)
                # ---- MM1 phase: h for all slots, 512 tokens ----
                h_t = {}
                for s, sl in enumerate(slots):
                    hdt = BF16 if sl[1] == "b" else FP8
                    h_sb = hpool.tile([P, HC, T2], hdt, tag=f"h{s}", name=f"h{s}")
                    h_t[s] = h_sb
                    if sl[0] == "b":
                        w1_tiles = []
                        for dc in range(DC):
                            wt = w1bp.tile([P, H], BF16, tag=f"w1b{dc}")
                            d = nc.sync.dma_start(
                                out=wt,
                                in_=dram["w1b"][i1[s], dc * P:(dc + 1) * P, :],
                            )
                            chain(d)
                            w1_tiles.append(wt)
                        prev_block[0] = d
                        for hc in range(HC):
                            p1 = ps1.tile([P, T2], F32, tag="p1")
                            for dc in range(DC):
                                nc.tensor.matmul(
                                    p1,
                                    w1_tiles[dc][:, hc * P:(hc + 1) * P],
                                    xb_t[dc][:, tsl],
                                    start=(dc == 0),
                                    stop=(dc == DC - 1),
                                )
                            nc.scalar.activation(
                                h_sb[:, hc, :],
                                p1,
                                mybir.ActivationFunctionType.Gelu,
                                bias=b1_sb[:, hc, s:s + 1],
                            )
                    else:
                        w1_tiles = []
                        for dq in range(DQ):
                            wt = w18p.tile([P, 2, H], FP8, tag=f"w18{dq}")
                            d = nc.sync.dma_start(out=wt, in_=dram["w18"][i1[s], dq])
                            chain(d)
                            w1_tiles.append(wt)
                        prev_block[0] = d
                        for hc in range(HC):
                            p1 = ps1.tile([P, T2], F32, tag="p1")
                            for dq in range(DQ):
                                nc.tensor.matmul(
                                    p1,
                                    w1_tiles[dq][:, :, hc * P:(hc + 1) * P],
                                    x8_t[dq][:, :, tsl],
                                    start=(dq == 0),
                                    stop=(dq == DQ - 1),
                                    perf_mode=mybir.MatmulPerfMode.DoubleRow,
                                )
                            nc.scalar.activation(
                                h_sb[:, hc, :],
                                p1,
                                mybir.ActivationFunctionType.Gelu,
                                bias=b1_sb[:, hc, s:s + 1],
                                scale=1.0 / W1_SCALE,
                            )

                # ---- MM2 phases: two column passes, accumulate over slots ----
                for kc in range(KC):
                    for si, (s, sl) in enumerate(list(enumerate(slots))):
                        first, last = si == 0, si == NS - 1
                        wdt = BF16 if sl[1] == "b" else FP8
                        arr = "w2b" if sl[1] == "b" else "w28"
                        w2t = w2p.tile([P, HC, KW], wdt, tag=f"w2{arr[-1]}")
                        d = nc.sync.dma_start(out=w2t, in_=dram[arr][i2[s], kc])
                        chain(d)
                        prev_block[0] = d
                        for tt in range(TT):
                            ttsl = slice(tt * P, (tt + 1) * P)
                            if sl[1] == "b":
                                for hc in range(HC):
                                    nc.tensor.matmul(
                                        pout[tt],
                                        h_t[s][:, hc, ttsl],
                                        w2t[:, hc, :],
                                        start=(first and hc == 0),
                                        stop=(last and hc == HC - 1),
                                    )
                            else:
                                for hq in range(HQ):
                                    nc.tensor.matmul(
                                        pout[tt],
                                        h_t[s][:, 2 * hq:2 * hq + 2, ttsl],
                                        w2t[:, 2 * hq:2 * hq + 2, :],
                                        start=(first and hq == 0),
                                        stop=(last and hq == HQ - 1),
                                        perf_mode=mybir.MatmulPerfMode.DoubleRow,
                                    )
                    ksl = slice(kc * KW, (kc + 1) * KW)
                    for tt in range(TT):
                        o_sb = outp.tile([P, KW], F32, tag="o")
                        nc.vector.scalar_tensor_tensor(
                            out=o_sb,
                            in0=pout[tt],
                            scalar=1.0 / C_W2,
                            in1=cbias_sb[:, ksl],
                            op0=mybir.AluOpType.mult,
                            op1=mybir.AluOpType.add,
                        )
                        nc.sync.dma_start(
                            out=out_r[:, half * TT + tt, ksl], in_=o_sb
                        )
    nc.finalize()
    return nc


_prog_cache = {}


def _get_program(slots, TC, D, H):
    key = (tuple(slots), TC, D, H)
    if key not in _prog_cache:
        _prog_cache[key] = build_program(list(slots), TC, D, H)
    return _prog_cache[key]


# precision rank: heavier-wbar experts go to lower rank (more precise) slots
_RANK = {"bb": 0, "b8": 1, "8b": 2, "88": 3}


def host_prep(op_logits, token_feats, W1, b1, W2, b2, slots):
    op_logits = np.asarray(op_logits, dtype=np.float32)
    token_feats = np.asarray(token_feats, dtype=np.float32)
    W1 = np.asarray(W1, dtype=np.float32)
    b1 = np.asarray(b1, dtype=np.float32)
    W2 = np.asarray(W2, dtype=np.float32)
    b2 = np.asarray(b2, dtype=np.float32)

    B, T, D = token_feats.shape
    V, _, H = W1.shape
    BT = B * T
    TC = BT // N_CORES
    NS = len(slots)
    assert NS == V
    DC, HC = D // P, H // P
    DQ = DC // 2
    KC, KW = 2, H // 2

    lg = op_logits.astype(np.float64)
    e = np.exp(lg - lg.max(axis=-1, keepdims=True))
    w = e / e.sum(axis=-1, keepdims=True)
    wbar = w.mean(axis=1)                    # [B, V], includes the 1/L
    cbias = wbar @ b2.astype(np.float64)     # [B, H]

    x_t = np.ascontiguousarray(token_feats.reshape(BT, D).T)  # [D, BT] f32

    # slot positions ordered most-precise first; position j gets the j-th
    # heaviest expert of the core's batch row
    slot_order = sorted(range(NS), key=lambda s: (_RANK[slots[s]], s))

    # per-batch prep (cores 2b and 2b+1 share everything except x/out)
    batch_data = []
    for bb in range(B):
        order = np.argsort(wbar[bb])[::-1]
        expert_of_slot = {}
        for j, pos in enumerate(slot_order):
            expert_of_slot[pos] = int(order[j])
        w1b, w18, w2b, w28 = [], [], [], []
        b1t = np.zeros((H, NS), dtype=np.float32)
        for s in range(NS):
            v = expert_of_slot[s]
            b1t[:, s] = b1[v]
            if slots[s][0] == "b":
                w1b.append(W1[v].astype(NP_BF16))
            else:
                # [D, H] -> [DQ, P, 2, H]
                w18.append(
                    (W1_SCALE * W1[v])
                    .astype(NP_FP8)
                    .reshape(DQ, 2, P, H)
                    .transpose(0, 2, 1, 3)
                )
            w2s = (C_W2 * wbar[bb, v]) * W2[v]  # [H, H]
            # [H, H] -> [KC, P, HC, KW]: element (kc,p,hc,k) = W2[hc*128+p, kc*KW+k]
            w2r = (
                w2s.reshape(HC, P, KC, KW).transpose(2, 1, 0, 3)
            )
            if slots[s][1] == "b":
                w2b.append(w2r.astype(NP_BF16))
            else:
                w28.append(w2r.astype(NP_FP8))
        dmap = {
            "b1t": b1t,
            "cbias": np.ascontiguousarray(
                np.broadcast_to(cbias[bb].astype(np.float32), (P, H))
            ),
        }
        if w1b:
            dmap["w1b"] = np.ascontiguousarray(np.stack(w1b))
        if w18:
            dmap["w18"] = np.ascontiguousarray(np.stack(w18))
        if w2b:
            dmap["w2b"] = np.ascontiguousarray(np.stack(w2b))
        if w28:
            dmap["w28"] = np.ascontiguousarray(np.stack(w28))
        batch_data.append(dmap)

    any_b = any(s[0] == "b" for s in slots)
    any_8 = any(s[0] == "8" for s in slots)
    in_maps = []
    for c in range(N_CORES):
        bc = (c * TC) // T
        xc = x_t[:, c * TC:(c + 1) * TC]
        m = dict(batch_data[bc])
        if any_b:
            m["xb"] = np.ascontiguousarray(xc.astype(NP_BF16))
        if any_8:
            # [D, TC] -> [DQ, P, 2, TC]
            m["x8"] = np.ascontiguousarray(
                xc.astype(NP_FP8).reshape(DQ, 2, P, TC).transpose(0, 2, 1, 3)
            )
        in_maps.append(m)
    return in_maps, (B, T, D, H, V, TC)


LAST_RESULTS = None


def kernel(op_logits, token_feats, W1, b1, W2, b2):
    global LAST_RESULTS
    slots = tuple(os.environ.get("MOE_SLOTS", DEFAULT_SLOTS).split(","))
    in_maps, (B, T, D, H, V, TC) = host_prep(
        op_logits, token_feats, W1, b1, W2, b2, slots
    )
    nc = _get_program(slots, TC, D, H)
    res = run_bass_kernel_spmd(
        nc,
        in_maps,
        list(range(N_CORES)),
        trace=os.environ.get("MOE_TRACE", "0") == "1",
    )
    LAST_RESULTS = res
    outs = [res.results[c]["out"] for c in range(N_CORES)]
    return np.concatenate(outs, axis=0).reshape(B, T, H).astype(np.float32)


# revision 6
# speedup vs baseline: 1.1843x; 1.0268x over previous
# MoE routing kernel for Trainium2 (Bass/Tile), SPMD over 8 NeuronCores.
#
# Reference computation (B=4, T=2048, D=H=1024, V=8, L=4):
#   h      = gelu(einsum("btd,vdh->btvh", X, W1) + b1)
#   outs   = einsum("btvh,vhk->btvk", h, W2) + b2
#   w      = softmax(op_logits, axis=-1)            # [B, L, V]
#   result = einsum("blv,btvh->bth", w, outs) / L
#
# Strategy (v2, mixed precision):
#   - Host: softmax + mean over L -> wbar[B, V]; fold b2 into cbias = wbar@b2.
#   - Data parallel over tokens: core c owns tokens [c*1024, (c+1)*1024), all
#     inside one batch row, so wbar/cbias are per-core constants.
#   - wbar is folded into W2 on host (W2 * wbar_v * 128), so MM2 accumulates
#     across ALL experts directly in PSUM; the single post-op per output tile
#     is out = psum/128 + cbias.
#   - Mixed precision expert slots: the program has a fixed list of 8 slots,
#     each (mm1, mm2) in {bf16, fp8e4}^2. fp8 matmuls use DoubleRow perf mode
#     (256-deep contraction per instruction = 2x bf16 throughput; verified on
#     HW). Per core, experts are assigned to slots by descending wbar (heavy
#     experts -> bf16 slots), which keeps the softmax-weighted quantization
#     error under the harness gate: sim says 5xbb+3x88 -> rel 1.84e-2.
#   - fp8 scaling: W1*32 quantized, descaled by the gelu activation's input
#     scale (1/32); X and h quantized at natural scale (subnormal loss of
#     tiny elements contributes negligibly to 1024-deep dot products).
#   - Loop structure per core: two 512-token halves; per half one MM1 phase
#     (h for all 8 slots resident in SBUF) then two MM2 column passes (kc)
#     accumulating over slots into 4 persistent PSUM banks; ps1 rotates in 3
#     other banks. Weights stream in consumption order on a single DMA chain.

import os

import numpy as np
import ml_dtypes

import concourse.bass as bass
import concourse.mybir as mybir
import concourse.tile as tile
from concourse import bacc
from concourse.bass_utils import run_bass_kernel_spmd

N_CORES = 8
P = 128
C_W2 = 128.0  # global PSUM scale carried by the folded W2
W1_SCALE = 32.0  # fp8 W1 pre-scale, descaled in the gelu activation

BF16 = mybir.dt.bfloat16
FP8 = mybir.dt.float8e4
F32 = mybir.dt.float32
NP_BF16 = ml_dtypes.bfloat16
NP_FP8 = ml_dtypes.float8_e4m3

DEFAULT_SLOTS = "88,88,88,bb,bb,bb,bb,bb"


def build_program(slots, TC, D, H):
    """One NeuronCore's program: TC tokens, len(slots) expert slots."""
    NS = len(slots)
    DC, HC = D // P, H // P
    DQ, HQ = DC // 2, HC // 2
    T2 = 512            # tokens per half
    NH = TC // T2       # halves
    TT = T2 // P        # 128-token tiles per half
    KC = 2              # 512-col MM2 passes
    KW = H // KC
    n1b = sum(s[0] == "b" for s in slots)
    n18 = NS - n1b
    n2b = sum(s[1] == "b" for s in slots)
    n28 = NS - n2b

    nc = bacc.Bacc(trn_type="TRN2")
    dram = {}
    if n1b:
        dram["xb"] = nc.declare_dram_parameter("xb", [D, TC], BF16, isOutput=False)
        dram["w1b"] = nc.declare_dram_parameter(
            "w1b", [n1b, D, H], BF16, isOutput=False
        )
    if n18:
        dram["x8"] = nc.declare_dram_parameter(
            "x8", [DQ, P, 2, TC], FP8, isOutput=False
        )
        dram["w18"] = nc.declare_dram_parameter(
            "w18", [n18, DQ, P, 2, H], FP8, isOutput=False
        )
    if n2b:
        dram["w2b"] = nc.declare_dram_parameter(
            "w2b", [n2b, KC, P, HC, KW], BF16, isOutput=False
        )
    if n28:
        dram["w28"] = nc.declare_dram_parameter(
            "w28", [n28, KC, P, HC, KW], FP8, isOutput=False
        )
    b1t = nc.declare_dram_parameter("b1t", [H, NS], F32, isOutput=False)
    cbias = nc.declare_dram_parameter("cbias", [P, H], F32, isOutput=False)
    out = nc.declare_dram_parameter("out", [TC, H], F32, isOutput=True)

    # per-slot index into the per-precision weight arrays
    i1 = {}
    c1b = c18 = 0
    for s, sl in enumerate(slots):
        if sl[0] == "b":
            i1[s] = c1b
            c1b += 1
        else:
            i1[s] = c18
            c18 += 1
    i2 = {}
    c2b = c28 = 0
    for s, sl in enumerate(slots):
        if sl[1] == "b":
            i2[s] = c2b
            c2b += 1
        else:
            i2[s] = c28
            c28 += 1

    with tile.TileContext(nc) as tc:
        with (
            tc.tile_pool(name="const", bufs=1) as cpool,
            tc.tile_pool(name="w1bp", bufs=2) as w1bp,
            tc.tile_pool(name="w18p", bufs=2) as w18p,
            tc.tile_pool(name="w2p", bufs=3) as w2p,
            tc.tile_pool(name="hbuf", bufs=1) as hpool,
            tc.tile_pool(name="outp", bufs=3) as outp,
            tc.tile_pool(name="ps1", bufs=3, space="PSUM") as ps1,
            tc.tile_pool(name="pout", bufs=1, space="PSUM") as poutp,
        ):
            b1_sb = cpool.tile([P, HC, NS], F32)
            nc.sync.dma_start(out=b1_sb, in_=b1t.rearrange("(hc p) v -> p hc v", p=P))
            cbias_sb = cpool.tile([P, H], F32)
            nc.sync.dma_start(out=cbias_sb, in_=cbias[:])

            # x8 tiles: unchained, startup-critical (the first slots are fp8),
            # stream at full fanout. xb is chained into the weight stream just
            # before the first bf16 slot's w1 (emitted lazily below) so it
            # doesn't steal startup bandwidth.
            xb_t, x8_t = [], []
            if n18:
                for dq in range(DQ):
                    t = cpool.tile([P, 2, TC], FP8, tag=f"x8{dq}", name=f"x8{dq}")
                    nc.sync.dma_start(out=t, in_=dram["x8"][dq])
                    x8_t.append(t)

            pout = [
                poutp.tile([P, KW], F32, tag=f"po{tt}", name=f"po{tt}")
                for tt in range(TT)
            ]
            out_r = out.rearrange("(tb p) k -> p tb k", p=P)

            # Weight blocks are chained (block granularity) so later transfers
            # can't steal bandwidth from the startup-critical x + first-slot
            # loads. Each block's DMAs depend on the previous block's last DMA.
            prev_block = [None]

            def chain(dma):
                if prev_block[0] is not None:
                    bass._add_dep_helper(
                        dma.ins, prev_block[0].ins, sync=True,
                        reason="serialize weight-block DMA issue",
                    )

            def ensure_xb():
                if n1b and not xb_t:
                    d = None
                    for dc in range(DC):
                        t = cpool.tile([P, TC], BF16, tag=f"xb{dc}", name=f"xb{dc}")
                        d = nc.sync.dma_start(
                            out=t, in_=dram["xb"][dc * P:(dc + 1) * P, :]
                        )
                        if n18:
                            chain(d)
                        xb_t.append(t)
                    if n18:
                        prev_block[0] = d

            for half in range(NH):
                tsl = slice(half * T2, (half + 1) * T2)
                # ---- MM1 phase: h for all slots, 512 tokens ----
                h_t = {}
                for s, sl in enumerate(slots):
                    hdt = BF16 if sl[1] == "b" else FP8
                    h_sb = hpool.tile([P, HC, T2], hdt, tag=f"h{s}", name=f"h{s}")
                    h_t[s] = h_sb
                    if sl[0] == "b":
                        ensure_xb()
                        w1_tiles = []
                        for dc in range(DC):
                            wt = w1bp.tile([P, H], BF16, tag=f"w1b{dc}")
                            d = nc.sync.dma_start(
                                out=wt,
                                in_=dram["w1b"][i1[s], dc * P:(dc + 1) * P, :],
                            )
                            chain(d)
                            w1_tiles.append(wt)
                        prev_block[0] = d
                        for hc in range(HC):
                            p1 = ps1.tile([P, T2], F32, tag="p1")
                            for dc in range(DC):
                                nc.tensor.matmul(
                                    p1,
                                    w1_tiles[dc][:, hc * P:(hc + 1) * P],
                                    xb_t[dc][:, tsl],
                                    start=(dc == 0),
                                    stop=(dc == DC - 1),
                                )
                            nc.scalar.activation(
                                h_sb[:, hc, :],
                                p1,
                                mybir.ActivationFunctionType.Gelu,
                                bias=b1_sb[:, hc, s:s + 1],
                            )
                    else:
                        w1_tiles = []
                        for dq in range(DQ):
                            wt = w18p.tile([P, 2, H], FP8, tag=f"w18{dq}")
                            d = nc.sync.dma_start(out=wt, in_=dram["w18"][i1[s], dq])
                            chain(d)
                            w1_tiles.append(wt)
                        prev_block[0] = d
                        for hc in range(HC):
                            p1 = ps1.tile([P, T2], F32, tag="p1")
                            for dq in range(DQ):
                                nc.tensor.matmul(
                                    p1,
                                    w1_tiles[dq][:, :, hc * P:(hc + 1) * P],
                                    x8_t[dq][:, :, tsl],
                                    start=(dq == 0),
                                    stop=(dq == DQ - 1),
                                    perf_mode=mybir.MatmulPerfMode.DoubleRow,
                                )
                            nc.scalar.activation(
                                h_sb[:, hc, :],
                                p1,
                                mybir.ActivationFunctionType.Gelu,
                                bias=b1_sb[:, hc, s:s + 1],
                                scale=1.0 / W1_SCALE,
                            )

                # ---- MM2 phases: two column passes, accumulate over slots ----
                for kc in range(KC):
                    ksl = slice(kc * KW, (kc + 1) * KW)
                    for si, (s, sl) in enumerate(list(enumerate(slots))):
                        first, last = si == 0, si == NS - 1
                        wdt = BF16 if sl[1] == "b" else FP8
                        arr = "w2b" if sl[1] == "b" else "w28"
                        w2t = w2p.tile([P, HC, KW], wdt, tag=f"w2{arr[-1]}")
                        d = nc.sync.dma_start(out=w2t, in_=dram[arr][i2[s], kc])
                        chain(d)
                        prev_block[0] = d
                        for tt in range(TT):
                            ttsl = slice(tt * P, (tt + 1) * P)
                            if sl[1] == "b":
                                for hc in range(HC):
                                    nc.tensor.matmul(
                                        pout[tt],
                                        h_t[s][:, hc, ttsl],
                                        w2t[:, hc, :],
                                        start=(first and hc == 0),
                                        stop=(last and hc == HC - 1),
                                    )
                            else:
                                for hq in range(HQ):
                                    nc.tensor.matmul(
                                        pout[tt],
                                        h_t[s][:, 2 * hq:2 * hq + 2, ttsl],
                                        w2t[:, 2 * hq:2 * hq + 2, :],
                                        start=(first and hq == 0),
                                        stop=(last and hq == HQ - 1),
                                        perf_mode=mybir.MatmulPerfMode.DoubleRow,
                                    )
                            if last:
                                # drain each tt as soon as its group closes so
                                # DVE/DMA overlap the remaining tts' matmuls
                                # and the next pass reopens banks without WAR
                                # stalls
                                o_sb = outp.tile([P, KW], F32, tag="o")
                                nc.vector.scalar_tensor_tensor(
                                    out=o_sb,
                                    in0=pout[tt],
                                    scalar=1.0 / C_W2,
                                    in1=cbias_sb[:, ksl],
                                    op0=mybir.AluOpType.mult,
                                    op1=mybir.AluOpType.add,
                                )
                                nc.sync.dma_start(
                                    out=out_r[:, half * TT + tt, ksl], in_=o_sb
                                )
    nc.finalize()
    return nc


_prog_cache = {}


def _get_program(slots, TC, D, H):
    key = (tuple(slots), TC, D, H)
    if key not in _prog_cache:
        _prog_cache[key] = build_program(list(slots), TC, D, H)
    return _prog_cache[key]


# precision rank: heavier-wbar experts go to lower rank (more precise) slots
_RANK = {"bb": 0, "b8": 1, "8b": 2, "88": 3}


def host_prep(op_logits, token_feats, W1, b1, W2, b2, slots):
    op_logits = np.asarray(op_logits, dtype=np.float32)
    token_feats = np.asarray(token_feats, dtype=np.float32)
    W1 = np.asarray(W1, dtype=np.float32)
    b1 = np.asarray(b1, dtype=np.float32)
    W2 = np.asarray(W2, dtype=np.float32)
    b2 = np.asarray(b2, dtype=np.float32)

    B, T, D = token_feats.shape
    V, _, H = W1.shape
    BT = B * T
    TC = BT // N_CORES
    NS = len(slots)
    assert NS == V
    DC, HC = D // P, H // P
    DQ = DC // 2
    KC, KW = 2, H // 2

    lg = op_logits.astype(np.float64)
    e = np.exp(lg - lg.max(axis=-1, keepdims=True))
    w = e / e.sum(axis=-1, keepdims=True)
    wbar = w.mean(axis=1)                    # [B, V], includes the 1/L
    cbias = wbar @ b2.astype(np.float64)     # [B, H]

    x_t = np.ascontiguousarray(token_feats.reshape(BT, D).T)  # [D, BT] f32

    # slot positions ordered most-precise first; position j gets the j-th
    # heaviest expert of the core's batch row
    slot_order = sorted(range(NS), key=lambda s: (_RANK[slots[s]], s))

    # per-batch prep (cores 2b and 2b+1 share everything except x/out)
    batch_data = []
    for bb in range(B):
        order = np.argsort(wbar[bb])[::-1]
        expert_of_slot = {}
        for j, pos in enumerate(slot_order):
            expert_of_slot[pos] = int(order[j])
        w1b, w18, w2b, w28 = [], [], [], []
        b1t = np.zeros((H, NS), dtype=np.float32)
        for s in range(NS):
            v = expert_of_slot[s]
            b1t[:, s] = b1[v]
            if slots[s][0] == "b":
                w1b.append(W1[v].astype(NP_BF16))
            else:
                # [D, H] -> [DQ, P, 2, H]
                w18.append(
                    (W1_SCALE * W1[v])
                    .astype(NP_FP8)
                    .reshape(DQ, 2, P, H)
                    .transpose(0, 2, 1, 3)
                )
            w2s = (C_W2 * wbar[bb, v]) * W2[v]  # [H, H]
            # [H, H] -> [KC, P, HC, KW]: element (kc,p,hc,k) = W2[hc*128+p, kc*KW+k]
            w2r = (
                w2s.reshape(HC, P, KC, KW).transpose(2, 1, 0, 3)
            )
            if slots[s][1] == "b":
                w2b.append(w2r.astype(NP_BF16))
            else:
                w28.append(w2r.astype(NP_FP8))
        dmap = {
            "b1t": b1t,
            "cbias": np.ascontiguousarray(
                np.broadcast_to(cbias[bb].astype(np.float32), (P, H))
            ),
        }
        if w1b:
            dmap["w1b"] = np.ascontiguousarray(np.stack(w1b))
        if w18:
            dmap["w18"] = np.ascontiguousarray(np.stack(w18))
        if w2b:
            dmap["w2b"] = np.ascontiguousarray(np.stack(w2b))
        if w28:
            dmap["w28"] = np.ascontiguousarray(np.stack(w28))
        batch_data.append(dmap)

    any_b = any(s[0] == "b" for s in slots)
    any_8 = any(s[0] == "8" for s in slots)
    in_maps = []
    for c in range(N_CORES):
        bc = (c * TC) // T
        xc = x_t[:, c * TC:(c + 1) * TC]
        m = dict(batch_data[bc])
        if any_b:
            m["xb"] = np.ascontiguousarray(xc.astype(NP_BF16))
        if any_8:
            # [D, TC] -> [DQ, P, 2, TC]
            m["x8"] = np.ascontiguousarray(
                xc.astype(NP_FP8).reshape(DQ, 2, P, TC).transpose(0, 2, 1, 3)
            )
        in_maps.append(m)
    return in_maps, (B, T, D, H, V, TC)


LAST_RESULTS = None


def kernel(op_logits, token_feats, W1, b1, W2, b2):
    global LAST_RESULTS
    slots = tuple(os.environ.get("MOE_SLOTS", DEFAULT_SLOTS).split(","))
    in_maps, (B, T, D, H, V, TC) = host_prep(
        op_logits, token_feats, W1, b1, W2, b2, slots
    )
    nc = _get_program(slots, TC, D, H)
    res = run_bass_kernel_spmd(
        nc,
        in_maps,
        list(range(N_CORES)),
        trace=os.environ.get("MOE_TRACE", "0") == "1",
    )
    LAST_RESULTS = res
    outs = [res.results[c]["out"] for c in range(N_CORES)]
    return np.concatenate(outs, axis=0).reshape(B, T, H).astype(np.float32)


# revision 13
# speedup vs baseline: 1.1992x; 1.0126x over previous
# MoE routing kernel for Trainium2 (Bass/Tile), SPMD over 8 NeuronCores.
#
# Reference computation (B=4, T=2048, D=H=1024, V=8, L=4):
#   h      = gelu(einsum("btd,vdh->btvh", X, W1) + b1)
#   outs   = einsum("btvh,vhk->btvk", h, W2) + b2
#   w      = softmax(op_logits, axis=-1)            # [B, L, V]
#   result = einsum("blv,btvh->bth", w, outs) / L
#
# Strategy (v2, mixed precision):
#   - Host: softmax + mean over L -> wbar[B, V]; fold b2 into cbias = wbar@b2.
#   - Data parallel over tokens: core c owns tokens [c*1024, (c+1)*1024), all
#     inside one batch row, so wbar/cbias are per-core constants.
#   - wbar is folded into W2 on host (W2 * wbar_v * 128), so MM2 accumulates
#     across ALL experts directly in PSUM; the single post-op per output tile
#     is out = psum/128 + cbias.
#   - Mixed precision expert slots: the program has a fixed list of 8 slots,
#     each (mm1, mm2) in {bf16, fp8e4}^2. fp8 matmuls use DoubleRow perf mode
#     (256-deep contraction per instruction = 2x bf16 throughput; verified on
#     HW). Per core, experts are assigned to slots by descending wbar (heavy
#     experts -> bf16 slots), which keeps the softmax-weighted quantization
#     error under the harness gate: sim says 5xbb+3x88 -> rel 1.84e-2.
#   - fp8 scaling: W1*32 quantized, descaled by the gelu activation's input
#     scale (1/32); X and h quantized at natural scale (subnormal loss of
#     tiny elements contributes negligibly to 1024-deep dot products).
#   - Loop structure per core: two 512-token halves; per half one MM1 phase
#     (h for all 8 slots resident in SBUF) then two MM2 column passes (kc)
#     accumulating over slots into 4 persistent PSUM banks; ps1 rotates in 3
#     other banks. Weights stream in consumption order on a single DMA chain.

import os

import numpy as np
import ml_dtypes

import concourse.bass as bass
import concourse.mybir as mybir
import concourse.tile as tile
from concourse import bacc
from concourse.bass_utils import run_bass_kernel_spmd

N_CORES = 8
P = 128
C_W2 = 128.0  # global PSUM scale carried by the folded W2
W1_SCALE = 32.0  # fp8 W1 pre-scale, descaled in the gelu activation

BF16 = mybir.dt.bfloat16
FP8 = mybir.dt.float8e4
F32 = mybir.dt.float32
NP_BF16 = ml_dtypes.bfloat16
NP_FP8 = ml_dtypes.float8_e4m3

DEFAULT_SLOTS = "88,88,88,bb,bb,bb,bb,bb"


def build_program(slots, TC, D, H):
    """One NeuronCore's program: TC tokens, len(slots) expert slots."""
    NS = len(slots)
    DC, HC = D // P, H // P
    DQ, HQ = DC // 2, HC // 2
    T2 = 512            # tokens per half
    NH = TC // T2       # halves
    TT = T2 // P        # 128-token tiles per half
    KC = 2              # 512-col MM2 passes
    KW = H // KC
    n1b = sum(s[0] == "b" for s in slots)
    n18 = NS - n1b
    n2b = sum(s[1] == "b" for s in slots)
    n28 = NS - n2b

    nc = bacc.Bacc(trn_type="TRN2")
    dram = {}
    if n1b:
        dram["xb"] = nc.declare_dram_parameter(
            "xb", [P, DC, TC], BF16, isOutput=False
        )
        dram["w1b"] = nc.declare_dram_parameter(
            "w1b", [n1b, P, DC, H], BF16, isOutput=False
        )
    if n18:
        dram["x8"] = nc.declare_dram_parameter(
            "x8", [P, DQ, 2, TC], FP8, isOutput=False
        )
        dram["w18"] = nc.declare_dram_parameter(
            "w18", [n18, P, DQ, 2, H], FP8, isOutput=False
        )
    if n2b:
        dram["w2b"] = nc.declare_dram_parameter(
            "w2b", [n2b, KC, P, HC, KW], BF16, isOutput=False
        )
    if n28:
        dram["w28"] = nc.declare_dram_parameter(
            "w28", [n28, KC, P, HC, KW], FP8, isOutput=False
        )
    b1t = nc.declare_dram_parameter("b1t", [H, NS], F32, isOutput=False)
    cbias = nc.declare_dram_parameter("cbias", [P, H], F32, isOutput=False)
    out = nc.declare_dram_parameter("out", [TC, H], F32, isOutput=True)

    # per-slot index into the per-precision weight arrays
    i1 = {}
    c1b = c18 = 0
    for s, sl in enumerate(slots):
        if sl[0] == "b":
            i1[s] = c1b
            c1b += 1
        else:
            i1[s] = c18
            c18 += 1
    i2 = {}
    c2b = c28 = 0
    for s, sl in enumerate(slots):
        if sl[1] == "b":
            i2[s] = c2b
            c2b += 1
        else:
            i2[s] = c28
            c28 += 1

    with tile.TileContext(nc) as tc:
        with (
            tc.tile_pool(name="const", bufs=1) as cpool,
            tc.tile_pool(name="w1bp", bufs=2) as w1bp,
            tc.tile_pool(name="w18p", bufs=2) as w18p,
            tc.tile_pool(name="w2p", bufs=3) as w2p,
            tc.tile_pool(name="hbuf", bufs=1) as hpool,
            tc.tile_pool(name="outp", bufs=3) as outp,
            tc.tile_pool(name="ps1", bufs=3, space="PSUM") as ps1,
            tc.tile_pool(name="pout", bufs=1, space="PSUM") as poutp,
        ):
            b1_sb = cpool.tile([P, HC, NS], F32)
            nc.sync.dma_start(out=b1_sb, in_=b1t.rearrange("(hc p) v -> p hc v", p=P))
            cbias_sb = cpool.tile([P, H], F32)
            nc.sync.dma_start(out=cbias_sb, in_=cbias[:])

            # x8: unchained, startup-critical (the first slots are fp8). xb is
            # chained into the weight stream just before the first bf16 slot's
            # w1 (emitted lazily below) so it doesn't steal startup bandwidth.
            xb_box, x8_t = [], None
            if n18:
                x8_t = cpool.tile([P, DQ, 2, TC], FP8, tag="x8", name="x8")
                nc.sync.dma_start(out=x8_t, in_=dram["x8"][:])

            pout = [
                poutp.tile([P, KW], F32, tag=f"po{tt}", name=f"po{tt}")
                for tt in range(TT)
            ]
            out_r = out.rearrange("(tb p) k -> p tb k", p=P)

            # Weight blocks are chained with DEPTH-2 deps (each block waits on
            # the block two back) — preserves rough priority order so later
            # transfers can't starve startup-critical ones, while keeping two
            # blocks in flight to hide the ~3us per-link issue latency of
            # strict serial chaining.
            chain_hist = []

            def chain(dma):
                if len(chain_hist) >= 2:
                    bass._add_dep_helper(
                        dma.ins, chain_hist[-2].ins, sync=True,
                        reason="depth-2 weight-stream ordering",
                    )
                chain_hist.append(dma)

            def ensure_xb():
                if n1b and not xb_box:
                    t = cpool.tile([P, DC, TC], BF16, tag="xb", name="xb")
                    d = nc.sync.dma_start(out=t, in_=dram["xb"][:])
                    if n18:
                        chain(d)
                    xb_box.append(t)

            for half in range(NH):
                tsl = slice(half * T2, (half + 1) * T2)
                # ---- MM1 phase: h for all slots, 512 tokens ----
                h_t = {}
                for s, sl in enumerate(slots):
                    hdt = BF16 if sl[1] == "b" else FP8
                    h_sb = hpool.tile([P, HC, T2], hdt, tag=f"h{s}", name=f"h{s}")
                    h_t[s] = h_sb
                    if sl[0] == "b":
                        ensure_xb()
                        wt = w1bp.tile([P, DC, H], BF16, tag="w1b")
                        chain(nc.sync.dma_start(out=wt, in_=dram["w1b"][i1[s]]))
                        for hc in range(HC):
                            p1 = ps1.tile([P, T2], F32, tag="p1")
                            hsl = slice(hc * P, (hc + 1) * P)
                            for dc in range(DC):
                                nc.tensor.matmul(
                                    p1,
                                    wt[:, dc, hsl],
                                    xb_box[0][:, dc, tsl],
                                    start=(dc == 0),
                                    stop=(dc == DC - 1),
                                )
                            nc.scalar.activation(
                                h_sb[:, hc, :],
                                p1,
                                mybir.ActivationFunctionType.Gelu,
                                bias=b1_sb[:, hc, s:s + 1],
                            )
                    else:
                        wt = w18p.tile([P, DQ, 2, H], FP8, tag="w18")
                        chain(nc.sync.dma_start(out=wt, in_=dram["w18"][i1[s]]))
                        for hc in range(HC):
                            p1 = ps1.tile([P, T2], F32, tag="p1")
                            hsl = slice(hc * P, (hc + 1) * P)
                            for dq in range(DQ):
                                nc.tensor.matmul(
                                    p1,
                                    wt[:, dq, :, hsl],
                                    x8_t[:, dq, :, tsl],
                                    start=(dq == 0),
                                    stop=(dq == DQ - 1),
                                    perf_mode=mybir.MatmulPerfMode.DoubleRow,
                                )
                            nc.scalar.activation(
                                h_sb[:, hc, :],
                                p1,
                                mybir.ActivationFunctionType.Gelu,
                                bias=b1_sb[:, hc, s:s + 1],
                                scale=1.0 / W1_SCALE,
                            )

                # ---- MM2 phases: two column passes, accumulate over slots ----
                for kc in range(KC):
                    ksl = slice(kc * KW, (kc + 1) * KW)
                    for si, (s, sl) in enumerate(list(enumerate(slots))):
                        first, last = si == 0, si == NS - 1
                        wdt = BF16 if sl[1] == "b" else FP8
                        arr = "w2b" if sl[1] == "b" else "w28"
                        w2t = w2p.tile([P, HC, KW], wdt, tag=f"w2{arr[-1]}")
                        chain(nc.sync.dma_start(out=w2t, in_=dram[arr][i2[s], kc]))
                        for tt in range(TT):
                            ttsl = slice(tt * P, (tt + 1) * P)
                            if sl[1] == "b":
                                for hc in range(HC):
                                    nc.tensor.matmul(
                                        pout[tt],
                                        h_t[s][:, hc, ttsl],
                                        w2t[:, hc, :],
                                        start=(first and hc == 0),
                                        stop=(last and hc == HC - 1),
                                    )
                            else:
                                for hq in range(HQ):
                                    nc.tensor.matmul(
                                        pout[tt],
                                        h_t[s][:, 2 * hq:2 * hq + 2, ttsl],
                                        w2t[:, 2 * hq:2 * hq + 2, :],
                                        start=(first and hq == 0),
                                        stop=(last and hq == HQ - 1),
                                        perf_mode=mybir.MatmulPerfMode.DoubleRow,
                                    )
                            if last:
                                # drain each tt as soon as its group closes so
                                # DVE/DMA overlap the remaining tts' matmuls
                                # and the next pass reopens banks without WAR
                                # stalls
                                o_sb = outp.tile([P, KW], F32, tag="o")
                                nc.vector.scalar_tensor_tensor(
                                    out=o_sb,
                                    in0=pout[tt],
                                    scalar=1.0 / C_W2,
                                    in1=cbias_sb[:, ksl],
                                    op0=mybir.AluOpType.mult,
                                    op1=mybir.AluOpType.add,
                                )
                                nc.sync.dma_start(
                                    out=out_r[:, half * TT + tt, ksl], in_=o_sb
                                )
    nc.finalize()
    return nc


_prog_cache = {}


def _get_program(slots, TC, D, H):
    key = (tuple(slots), TC, D, H)
    if key not in _prog_cache:
        _prog_cache[key] = build_program(list(slots), TC, D, H)
    return _prog_cache[key]


# precision rank: heavier-wbar experts go to lower rank (more precise) slots
_RANK = {"bb": 0, "b8": 1, "8b": 2, "88": 3}


def host_prep(op_logits, token_feats, W1, b1, W2, b2, slots):
    op_logits = np.asarray(op_logits, dtype=np.float32)
    token_feats = np.asarray(token_feats, dtype=np.float32)
    W1 = np.asarray(W1, dtype=np.float32)
    b1 = np.asarray(b1, dtype=np.float32)
    W2 = np.asarray(W2, dtype=np.float32)
    b2 = np.asarray(b2, dtype=np.float32)

    B, T, D = token_feats.shape
    V, _, H = W1.shape
    BT = B * T
    TC = BT // N_CORES
    NS = len(slots)
    assert NS == V
    DC, HC = D // P, H // P
    DQ = DC // 2
    KC, KW = 2, H // 2

    lg = op_logits.astype(np.float64)
    e = np.exp(lg - lg.max(axis=-1, keepdims=True))
    w = e / e.sum(axis=-1, keepdims=True)
    wbar = w.mean(axis=1)                    # [B, V], includes the 1/L
    cbias = wbar @ b2.astype(np.float64)     # [B, H]

    x_t = np.ascontiguousarray(token_feats.reshape(BT, D).T)  # [D, BT] f32

    # slot positions ordered most-precise first; position j gets the j-th
    # heaviest expert of the core's batch row
    slot_order = sorted(range(NS), key=lambda s: (_RANK[slots[s]], s))

    # per-batch prep (cores 2b and 2b+1 share everything except x/out)
    batch_data = []
    for bb in range(B):
        order = np.argsort(wbar[bb])[::-1]
        expert_of_slot = {}
        for j, pos in enumerate(slot_order):
            expert_of_slot[pos] = int(order[j])
        w1b, w18, w2b, w28 = [], [], [], []
        b1t = np.zeros((H, NS), dtype=np.float32)
        for s in range(NS):
            v = expert_of_slot[s]
            b1t[:, s] = b1[v]
            if slots[s][0] == "b":
                # [D, H] -> [P, DC, H]
                w1b.append(
                    W1[v].astype(NP_BF16).reshape(DC, P, H).transpose(1, 0, 2)
                )
            else:
                # [D, H] -> [P, DQ, 2, H]
                w18.append(
                    (W1_SCALE * W1[v])
                    .astype(NP_FP8)
                    .reshape(DQ, 2, P, H)
                    .transpose(2, 0, 1, 3)
                )
            w2s = (C_W2 * wbar[bb, v]) * W2[v]  # [H, H]
            # [H, H] -> [KC, P, HC, KW]: element (kc,p,hc,k) = W2[hc*128+p, kc*KW+k]
            w2r = (
                w2s.reshape(HC, P, KC, KW).transpose(2, 1, 0, 3)
            )
            if slots[s][1] == "b":
                w2b.append(w2r.astype(NP_BF16))
            else:
                w28.append(w2r.astype(NP_FP8))
        dmap = {
            "b1t": b1t,
            "cbias": np.ascontiguousarray(
                np.broadcast_to(cbias[bb].astype(np.float32), (P, H))
            ),
        }
        if w1b:
            dmap["w1b"] = np.ascontiguousarray(np.stack(w1b))
        if w18:
            dmap["w18"] = np.ascontiguousarray(np.stack(w18))
        if w2b:
            dmap["w2b"] = np.ascontiguousarray(np.stack(w2b))
        if w28:
            dmap["w28"] = np.ascontiguousarray(np.stack(w28))
        batch_data.append(dmap)

    any_b = any(s[0] == "b" for s in slots)
    any_8 = any(s[0] == "8" for s in slots)
    in_maps = []
    for c in range(N_CORES):
        bc = (c * TC) // T
        xc = x_t[:, c * TC:(c + 1) * TC]
        m = dict(batch_data[bc])
        if any_b:
            # [D, TC] -> [P, DC, TC]
            m["xb"] = np.ascontiguousarray(
                xc.astype(NP_BF16).reshape(DC, P, TC).transpose(1, 0, 2)
            )
        if any_8:
            # [D, TC] -> [P, DQ, 2, TC]
            m["x8"] = np.ascontiguousarray(
                xc.astype(NP_FP8).reshape(DQ, 2, P, TC).transpose(2, 0, 1, 3)
            )
        in_maps.append(m)
    return in_maps, (B, T, D, H, V, TC)


LAST_RESULTS = None


def kernel(op_logits, token_feats, W1, b1, W2, b2):
    global LAST_RESULTS
    slots = tuple(os.environ.get("MOE_SLOTS", DEFAULT_SLOTS).split(","))
    in_maps, (B, T, D, H, V, TC) = host_prep(
        op_logits, token_feats, W1, b1, W2, b2, slots
    )
    nc = _get_program(slots, TC, D, H)
    res = run_bass_kernel_spmd(
        nc,
        in_maps,
        list(range(N_CORES)),
        trace=os.environ.get("MOE_TRACE", "0") == "1",
    )
    LAST_RESULTS = res
    outs = [res.results[c]["out"] for c in range(N_CORES)]
    return np.concatenate(outs, axis=0).reshape(B, T, H).astype(np.float32)


# revision 29
# speedup vs baseline: 1.2237x; 1.0205x over previous
# MoE routing kernel for Trainium2 (Bass/Tile), SPMD over 8 NeuronCores.
#
# Reference computation (B=4, T=2048, D=H=1024, V=8, L=4):
#   h      = gelu(einsum("btd,vdh->btvh", X, W1) + b1)
#   outs   = einsum("btvh,vhk->btvk", h, W2) + b2
#   w      = softmax(op_logits, axis=-1)            # [B, L, V]
#   result = einsum("blv,btvh->bth", w, outs) / L
#
# Strategy (v2, mixed precision):
#   - Host: softmax + mean over L -> wbar[B, V]; fold b2 into cbias = wbar@b2.
#   - Data parallel over tokens: core c owns tokens [c*1024, (c+1)*1024), all
#     inside one batch row, so wbar/cbias are per-core constants.
#   - wbar is folded into W2 on host (W2 * wbar_v * 128), so MM2 accumulates
#     across ALL experts directly in PSUM; the single post-op per output tile
#     is out = psum/128 + cbias.
#   - Mixed precision expert slots: the program has a fixed list of 8 slots,
#     each (mm1, mm2) in {bf16, fp8e4}^2. fp8 matmuls use DoubleRow perf mode
#     (256-deep contraction per instruction = 2x bf16 throughput; verified on
#     HW). Per core, experts are assigned to slots by descending wbar (heavy
#     experts -> bf16 slots), which keeps the softmax-weighted quantization
#     error under the harness gate: sim says 5xbb+3x88 -> rel 1.84e-2.
#   - fp8 scaling: W1*32 quantized, descaled by the gelu activation's input
#     scale (1/32); X and h quantized at natural scale (subnormal loss of
#     tiny elements contributes negligibly to 1024-deep dot products).
#   - Loop structure per core: two 512-token halves; per half one MM1 phase
#     (h for all 8 slots resident in SBUF) then two MM2 column passes (kc)
#     accumulating over slots into 4 persistent PSUM banks; ps1 rotates in 3
#     other banks. Weights stream in consumption order on a single DMA chain.

import os

import numpy as np
import ml_dtypes

import concourse.bass as bass
import concourse.mybir as mybir
import concourse.tile as tile
from concourse import bacc
from concourse.bass_utils import run_bass_kernel_spmd

N_CORES = 8
P = 128
C_W2 = 128.0  # global PSUM scale carried by the folded W2
W1_SCALE = 32.0  # fp8 W1 pre-scale, descaled in the gelu activation

BF16 = mybir.dt.bfloat16
FP8 = mybir.dt.float8e4
F32 = mybir.dt.float32
NP_BF16 = ml_dtypes.bfloat16
NP_FP8 = ml_dtypes.float8_e4m3

DEFAULT_SLOTS = "88,88,88,88,b8,bb,bb,bb"


def build_program(slots, TC, D, H, fold):
    """One NeuronCore's program: TC tokens, len(slots) expert slots.

    fold=True: for every mm2-fp8 slot the MM2 input is the gelu nonlinear
    remainder r = gelu(z) - a1*pre (per-unit best linear coefficient a1,
    shipped as na1 = -a1*mm1scale); the linear part rides exactly through one
    extra bf16 matmul X @ M with M = sum_folded W1*diag(a1)*W2' built on host.
    r is ~2x smaller than h, halving fp8 quantization error of those slots.
    """
    NS = len(slots)
    DC, HC = D // P, H // P
    DQ, HQ = DC // 2, HC // 2
    T2 = 512            # tokens per half
    NH = TC // T2       # halves
    TT = T2 // P        # 128-token tiles per half
    KC = 2              # 512-col MM2 passes
    KW = H // KC
    n1b = sum(s[0] == "b" for s in slots)
    n18 = NS - n1b
    n2b = sum(s[1] == "b" for s in slots)
    n28 = NS - n2b
    fold = fold and n28 > 0
    assert not fold or n1b > 0, "folding needs bf16 X on device"

    nc = bacc.Bacc(trn_type="TRN2")
    dram = {}
    if n1b:
        dram["xb"] = nc.declare_dram_parameter(
            "xb", [P, DC, TC], BF16, isOutput=False
        )
        dram["w1b"] = nc.declare_dram_parameter(
            "w1b", [n1b, P, DC, H], BF16, isOutput=False
        )
    if n18:
        dram["x8"] = nc.declare_dram_parameter(
            "x8", [P, DQ, 2, TC], FP8, isOutput=False
        )
        dram["w18"] = nc.declare_dram_parameter(
            "w18", [n18, P, DQ, 2, H], FP8, isOutput=False
        )
    if n2b:
        dram["w2b"] = nc.declare_dram_parameter(
            "w2b", [n2b, KC, P, HC, KW], BF16, isOutput=False
        )
    if n28:
        dram["w28"] = nc.declare_dram_parameter(
            "w28", [n28, KC, P, HC, KW], FP8, isOutput=False
        )
    b1t = nc.declare_dram_parameter("b1t", [H, NS], F32, isOutput=False)
    cbias = nc.declare_dram_parameter("cbias", [P, H], F32, isOutput=False)
    out = nc.declare_dram_parameter("out", [TC, H], F32, isOutput=True)
    if fold:
        dram["na1"] = nc.declare_dram_parameter("na1", [H, NS], F32, isOutput=False)
        dram["m"] = nc.declare_dram_parameter(
            "m", [KC, P, DC, KW], BF16, isOutput=False
        )

    # per-slot index into the per-precision weight arrays
    i1 = {}
    c1b = c18 = 0
    for s, sl in enumerate(slots):
        if sl[0] == "b":
            i1[s] = c1b
            c1b += 1
        else:
            i1[s] = c18
            c18 += 1
    i2 = {}
    c2b = c28 = 0
    for s, sl in enumerate(slots):
        if sl[1] == "b":
            i2[s] = c2b
            c2b += 1
        else:
            i2[s] = c28
            c28 += 1

    with tile.TileContext(nc) as tc:
        with (
            tc.tile_pool(name="const", bufs=1) as cpool,
            tc.tile_pool(name="w1bp", bufs=2) as w1bp,
            tc.tile_pool(name="w18p", bufs=2) as w18p,
            tc.tile_pool(name="w2p", bufs=4) as w2p,
            tc.tile_pool(name="hbuf", bufs=1) as hpool,
            tc.tile_pool(name="gbuf", bufs=2) as gpool,
            tc.tile_pool(name="outp", bufs=3) as outp,
            tc.tile_pool(name="ps1", bufs=4, space="PSUM") as ps1,
            tc.tile_pool(name="pout", bufs=1, space="PSUM") as poutp,
        ):
            b1_sb = cpool.tile([P, HC, NS], F32)
            nc.sync.dma_start(out=b1_sb, in_=b1t.rearrange("(hc p) v -> p hc v", p=P))
            cbias_sb = cpool.tile([P, H], F32)
            nc.sync.dma_start(out=cbias_sb, in_=cbias[:])
            na1_sb = None
            if fold:
                na1_sb = cpool.tile([P, HC, NS], F32, tag="na1", name="na1")
                nc.sync.dma_start(
                    out=na1_sb, in_=dram["na1"].rearrange("(hc p) v -> p hc v", p=P)
                )

            # x8: unchained, startup-critical (the first slots are fp8). xb is
            # chained into the weight stream just before the first bf16 slot's
            # w1 (emitted lazily below) so it doesn't steal startup bandwidth.
            xb_box, x8_t = [], None
            if n18:
                x8_t = cpool.tile([P, DQ, 2, TC], FP8, tag="x8", name="x8")
                nc.sync.dma_start(out=x8_t, in_=dram["x8"][:])

            pout = [
                poutp.tile([P, KW], F32, tag=f"po{tt}", name=f"po{tt}")
                for tt in range(TT)
            ]
            m_tiles = {}
            out_r = out.rearrange("(tb p) k -> p tb k", p=P)

            # Weight blocks are chained with DEPTH-2 deps (each block waits on
            # the block two back) — preserves rough priority order so later
            # transfers can't starve startup-critical ones, while keeping two
            # blocks in flight to hide the ~3us per-link issue latency of
            # strict serial chaining.
            chain_hist = []   # w1 + x stream
            chain2_hist = []  # w2 + M stream (kept off the startup path)

            def _chain(hist, dma):
                if len(hist) >= 2:
                    bass._add_dep_helper(
                        dma.ins, hist[-2].ins, sync=True,
                        reason="depth-2 weight-stream ordering",
                    )
                hist.append(dma)

            def chain(dma):
                _chain(chain_hist, dma)

            def chain2(dma):
                if not chain2_hist and chain_hist:
                    # seed so the w2 stream starts only after half-0's w1 set
                    chain2_hist.extend([chain_hist[-1]] * 2)
                _chain(chain2_hist, dma)

            def ensure_xb():
                if n1b and not xb_box:
                    t = cpool.tile([P, DC, TC], BF16, tag="xb", name="xb")
                    d = nc.sync.dma_start(out=t, in_=dram["xb"][:])
                    if n18:
                        chain(d)
                    xb_box.append(t)

            for half in range(NH):
                tsl = slice(half * T2, (half + 1) * T2)
                # ---- MM1 phase: h for all slots, 512 tokens ----
                h_t = {}
                for s, sl in enumerate(slots):
                    hdt = BF16 if sl[1] == "b" else FP8
                    h_sb = hpool.tile([P, HC, T2], hdt, tag=f"h{s}", name=f"h{s}")
                    h_t[s] = h_sb
                    if sl[0] == "b":
                        ensure_xb()
                        wt = w1bp.tile([P, DC, H], BF16, tag="w1b")
                        chain(nc.sync.dma_start(out=wt, in_=dram["w1b"][i1[s]]))
                        for hc in range(HC):
                            p1 = ps1.tile([P, T2], F32, tag="p1")
                            hsl = slice(hc * P, (hc + 1) * P)
                            for dc in range(DC):
                                nc.tensor.matmul(
                                    p1,
                                    wt[:, dc, hsl],
                                    xb_box[0][:, dc, tsl],
                                    start=(dc == 0),
                                    stop=(dc == DC - 1),
                                )
                            if fold and sl[1] == "8":
                                g_sb = gpool.tile([P, T2], F32, tag="g")
                                nc.scalar.activation(
                                    g_sb,
                                    p1,
                                    mybir.ActivationFunctionType.Gelu,
                                    bias=b1_sb[:, hc, s:s + 1],
                                )
                                nc.vector.scalar_tensor_tensor(
                                    out=h_sb[:, hc, :],
                                    in0=p1,
                                    scalar=na1_sb[:, hc, s:s + 1],
                                    in1=g_sb,
                                    op0=mybir.AluOpType.mult,
                                    op1=mybir.AluOpType.add,
                                )
                            else:
                                nc.scalar.activation(
                                    h_sb[:, hc, :],
                                    p1,
                                    mybir.ActivationFunctionType.Gelu,
                                    bias=b1_sb[:, hc, s:s + 1],
                                )
                    else:
                        wt = w18p.tile([P, DQ, 2, H], FP8, tag="w18")
                        chain(nc.sync.dma_start(out=wt, in_=dram["w18"][i1[s]]))
                        for hc in range(HC):
                            p1 = ps1.tile([P, T2], F32, tag="p1")
                            hsl = slice(hc * P, (hc + 1) * P)
                            for dq in range(DQ):
                                nc.tensor.matmul(
                                    p1,
                                    wt[:, dq, :, hsl],
                                    x8_t[:, dq, :, tsl],
                                    start=(dq == 0),
                                    stop=(dq == DQ - 1),
                                    perf_mode=mybir.MatmulPerfMode.DoubleRow,
                                )
                            if fold and sl[1] == "8":
                                g_sb = gpool.tile([P, T2], F32, tag="g")
                                nc.scalar.activation(
                                    g_sb,
                                    p1,
                                    mybir.ActivationFunctionType.Gelu,
                                    bias=b1_sb[:, hc, s:s + 1],
                                    scale=1.0 / W1_SCALE,
                                )
                                nc.vector.scalar_tensor_tensor(
                                    out=h_sb[:, hc, :],
                                    in0=p1,
                                    scalar=na1_sb[:, hc, s:s + 1],
                                    in1=g_sb,
                                    op0=mybir.AluOpType.mult,
                                    op1=mybir.AluOpType.add,
                                )
                            else:
                                nc.scalar.activation(
                                    h_sb[:, hc, :],
                                    p1,
                                    mybir.ActivationFunctionType.Gelu,
                                    bias=b1_sb[:, hc, s:s + 1],
                                    scale=1.0 / W1_SCALE,
                                )

                # ---- MM2 phases: two column passes, accumulate over slots ----
                for kc in range(KC):
                    ksl = slice(kc * KW, (kc + 1) * KW)
                    if fold:
                        # linear-part pseudo-slot: pout[tt] += X @ M[kc];
                        # opens the accumulation groups
                        if half == 0:
                            mt = cpool.tile(
                                [P, DC, KW], BF16, tag=f"m{kc}", name=f"m{kc}"
                            )
                            chain2(nc.sync.dma_start(out=mt, in_=dram["m"][kc]))
                            m_tiles[kc] = mt
                        mt = m_tiles[kc]
                        for tt in range(TT):
                            xsl = slice(half * T2 + tt * P, half * T2 + (tt + 1) * P)
                            for dc in range(DC):
                                nc.tensor.matmul(
                                    pout[tt],
                                    xb_box[0][:, dc, xsl],
                                    mt[:, dc, :],
                                    start=(dc == 0),
                                    stop=False,
                                )
                    for si, (s, sl) in enumerate(list(enumerate(slots))):
                        first, last = (si == 0 and not fold), si == NS - 1
                        wdt = BF16 if sl[1] == "b" else FP8
                        arr = "w2b" if sl[1] == "b" else "w28"
                        w2t = w2p.tile([P, HC, KW], wdt, tag=f"w2{arr[-1]}")
                        chain2(nc.sync.dma_start(out=w2t, in_=dram[arr][i2[s], kc]))
                        for tt in range(TT):
                            ttsl = slice(tt * P, (tt + 1) * P)
                            if sl[1] == "b":
                                for hc in range(HC):
                                    nc.tensor.matmul(
                                        pout[tt],
                                        h_t[s][:, hc, ttsl],
                                        w2t[:, hc, :],
                                        start=(first and hc == 0),
                                        stop=(last and hc == HC - 1),
                                    )
                            else:
                                for hq in range(HQ):
                                    nc.tensor.matmul(
                                        pout[tt],
                                        h_t[s][:, 2 * hq:2 * hq + 2, ttsl],
                                        w2t[:, 2 * hq:2 * hq + 2, :],
                                        start=(first and hq == 0),
                                        stop=(last and hq == HQ - 1),
                                        perf_mode=mybir.MatmulPerfMode.DoubleRow,
                                    )
                            if last:
                                # drain each tt as soon as its group closes so
                                # DVE/DMA overlap the remaining tts' matmuls
                                # and the next pass reopens banks without WAR
                                # stalls
                                o_sb = outp.tile([P, KW], F32, tag="o")
                                nc.vector.scalar_tensor_tensor(
                                    out=o_sb,
                                    in0=pout[tt],
                                    scalar=1.0 / C_W2,
                                    in1=cbias_sb[:, ksl],
                                    op0=mybir.AluOpType.mult,
                                    op1=mybir.AluOpType.add,
                                )
                                nc.sync.dma_start(
                                    out=out_r[:, half * TT + tt, ksl], in_=o_sb
                                )
    nc.finalize()
    return nc


_prog_cache = {}


def _get_program(slots, TC, D, H, fold):
    key = (tuple(slots), TC, D, H, fold)
    if key not in _prog_cache:
        _prog_cache[key] = build_program(list(slots), TC, D, H, fold)
    return _prog_cache[key]


def _gelu_lin_coeff(mu, sig):
    """Best linear coefficient a1 of gelu(z), z ~ N(mu, sig^2), elementwise.

    a1 = E[gelu'(z)] (Stein); 61-pt Gauss-Hermite over the actual b1/sigma.
    """
    from numpy.polynomial.hermite_e import hermegauss

    gx, gw = hermegauss(61)
    gw = gw / np.sqrt(2.0 * np.pi)
    z = mu[None, ...] + sig[None, ...] * gx[:, None, None]
    phi = np.exp(-0.5 * z * z) / np.sqrt(2.0 * np.pi)
    try:
        from scipy.special import erf as _erf

        Phi = 0.5 * (1.0 + _erf(z / np.sqrt(2.0)))
    except ImportError:
        import math

        Phi = 0.5 * (
            1.0 + np.frompyfunc(math.erf, 1, 1)(z / np.sqrt(2.0)).astype(np.float64)
        )
    return np.einsum("q,qvh->vh", gw, Phi + z * phi)


# precision rank: heavier-wbar experts go to lower rank (more precise) slots
_RANK = {"bb": 0, "b8": 1, "8b": 2, "88": 3}


def host_prep(op_logits, token_feats, W1, b1, W2, b2, slots, fold):
    op_logits = np.asarray(op_logits, dtype=np.float32)
    token_feats = np.asarray(token_feats, dtype=np.float32)
    W1 = np.asarray(W1, dtype=np.float32)
    b1 = np.asarray(b1, dtype=np.float32)
    W2 = np.asarray(W2, dtype=np.float32)
    b2 = np.asarray(b2, dtype=np.float32)

    B, T, D = token_feats.shape
    V, _, H = W1.shape
    BT = B * T
    TC = BT // N_CORES
    NS = len(slots)
    assert NS == V
    DC, HC = D // P, H // P
    DQ = DC // 2
    KC, KW = 2, H // 2

    lg = op_logits.astype(np.float64)
    e = np.exp(lg - lg.max(axis=-1, keepdims=True))
    w = e / e.sum(axis=-1, keepdims=True)
    wbar = w.mean(axis=1)                    # [B, V], includes the 1/L
    cbias = wbar @ b2.astype(np.float64)     # [B, H]

    x_t = np.ascontiguousarray(token_feats.reshape(BT, D).T)  # [D, BT] f32

    fold = fold and any(s[1] == "8" for s in slots)
    a1 = None
    P_lin = {}  # expert -> (W1*diag(a1)) @ W2, batch-independent
    if fold:
        sig = np.sqrt((W1.astype(np.float64) ** 2).sum(axis=1))  # [V, H]
        a1 = _gelu_lin_coeff(b1.astype(np.float64), sig)         # [V, H]

    # slot positions ordered most-precise first; position j gets the j-th
    # heaviest expert of the core's batch row
    slot_order = sorted(range(NS), key=lambda s: (_RANK[slots[s]], s))

    # per-batch prep (cores 2b and 2b+1 share everything except x/out)
    batch_data = []
    for bb in range(B):
        order = np.argsort(wbar[bb])[::-1]
        expert_of_slot = {}
        for j, pos in enumerate(slot_order):
            expert_of_slot[pos] = int(order[j])
        w1b, w18, w2b, w28 = [], [], [], []
        b1t = np.zeros((H, NS), dtype=np.float32)
        na1 = np.zeros((H, NS), dtype=np.float32)
        M = np.zeros((D, H), dtype=np.float32) if fold else None
        for s in range(NS):
            v = expert_of_slot[s]
            b1t[:, s] = b1[v]
            if fold and slots[s][1] == "8":
                na1[:, s] = -a1[v] * (1.0 if slots[s][0] == "b" else 1.0 / W1_SCALE)
                if v not in P_lin:
                    P_lin[v] = (
                        W1[v].astype(np.float32) * a1[v][None, :].astype(np.float32)
                    ) @ W2[v].astype(np.float32)
                M += np.float32(C_W2 * wbar[bb, v]) * P_lin[v]
            if slots[s][0] == "b":
                # [D, H] -> [P, DC, H]
                w1b.append(
                    W1[v].astype(NP_BF16).reshape(DC, P, H).transpose(1, 0, 2)
                )
            else:
                # [D, H] -> [P, DQ, 2, H]
                w18.append(
                    (W1_SCALE * W1[v])
                    .astype(NP_FP8)
                    .reshape(DQ, 2, P, H)
                    .transpose(2, 0, 1, 3)
                )
            w2s = (C_W2 * wbar[bb, v]) * W2[v]  # [H, H]
            # [H, H] -> [KC, P, HC, KW]: element (kc,p,hc,k) = W2[hc*128+p, kc*KW+k]
            w2r = (
                w2s.reshape(HC, P, KC, KW).transpose(2, 1, 0, 3)
            )
            if slots[s][1] == "b":
                w2b.append(w2r.astype(NP_BF16))
            else:
                w28.append(w2r.astype(NP_FP8))
        dmap = {
            "b1t": b1t,
            "cbias": np.ascontiguousarray(
                np.broadcast_to(cbias[bb].astype(np.float32), (P, H))
            ),
        }
        if w1b:
            dmap["w1b"] = np.ascontiguousarray(np.stack(w1b))
        if w18:
            dmap["w18"] = np.ascontiguousarray(np.stack(w18))
        if w2b:
            dmap["w2b"] = np.ascontiguousarray(np.stack(w2b))
        if w28:
            dmap["w28"] = np.ascontiguousarray(np.stack(w28))
        if fold:
            dmap["na1"] = na1
            # [D, H] -> [KC, P, DC, KW]
            dmap["m"] = np.ascontiguousarray(
                M.astype(NP_BF16).reshape(DC, P, KC, KW).transpose(2, 1, 0, 3)
            )
        batch_data.append(dmap)

    any_b = any(s[0] == "b" for s in slots)
    any_8 = any(s[0] == "8" for s in slots)
    in_maps = []
    for c in range(N_CORES):
        bc = (c * TC) // T
        xc = x_t[:, c * TC:(c + 1) * TC]
        m = dict(batch_data[bc])
        if any_b:
            # [D, TC] -> [P, DC, TC]
            m["xb"] = np.ascontiguousarray(
                xc.astype(NP_BF16).reshape(DC, P, TC).transpose(1, 0, 2)
            )
        if any_8:
            # [D, TC] -> [P, DQ, 2, TC]
            m["x8"] = np.ascontiguousarray(
                xc.astype(NP_FP8).reshape(DQ, 2, P, TC).transpose(2, 0, 1, 3)
            )
        in_maps.append(m)
    return in_maps, (B, T, D, H, V, TC)


LAST_RESULTS = None


def kernel(op_logits, token_feats, W1, b1, W2, b2):
    global LAST_RESULTS
    slots = tuple(os.environ.get("MOE_SLOTS", DEFAULT_SLOTS).split(","))
    fold = os.environ.get("MOE_FOLD", "1") == "1" and any(s[1] == "8" for s in slots)
    in_maps, (B, T, D, H, V, TC) = host_prep(
        op_logits, token_feats, W1, b1, W2, b2, slots, fold
    )
    nc = _get_program(slots, TC, D, H, fold)
    res = run_bass_kernel_spmd(
        nc,
        in_maps,
        list(range(N_CORES)),
        trace=os.environ.get("MOE_TRACE", "0") == "1",
    )
    LAST_RESULTS = res
    outs = [res.results[c]["out"] for c in range(N_CORES)]
    return np.concatenate(outs, axis=0).reshape(B, T, H).astype(np.float32)
